# revision 1
# baseline (speedup 1.0000x reference)
"""CapsuleConv2d (3-iteration dynamic routing) Bass kernel for 8 TRN2 cores.

Strategy (data-parallel over batch, 2 images per core):
  - priors[l, ij, o, u, f] computed by PE per 128-location tile:
    stationary = padded-x window [32=(f,d), 128 locs], moving = structured
    weight constants [32, (o,u,f)] per kernel tap ij.  s0 = 0.25*sum_r priors
    accumulated by PE in the same pass.
  - routing in "natural" layout (locations on partitions): DVE does the
    broadcast-multiplies + segmented reduces, ACT does exp/square/sqrt and
    PSUM->SBUF copies, PE transposes the final [128 locs, 32 ch] result for
    channel-major DMA out.
"""
import numpy as np

import concourse.bass as bass
import concourse.bacc as bacc
import concourse.tile as tile
import concourse.mybir as mybir
import concourse.bass_utils as bass_utils

# All ACT functions we use (Exp, Ln, Square, Copy, ...) live together in the
# "natural_log_exp_and_others" table set, but bacc's table-load pass picks a
# per-function set greedily (Ln -> natural_log, Exp -> exp_and_others),
# thrashing ~2.7us table loads between them.  Restrict Exp/Ln to the combined
# set so a single load covers the whole kernel.
_orig_get_tables = bacc.get_activation_tables
_AFT = mybir.ActivationFunctionType


def _patched_get_tables(arch):
    tables = dict(_orig_get_tables(arch))
    for name, funcs in tables.items():
        if name != "natural_log_exp_and_others":
            tables[name] = funcs - {_AFT.Exp, _AFT.Ln}
    return tables


bacc.get_activation_tables = _patched_get_tables

# ---- problem constants (hardcoded; must match setup_inputs) ----
O, F, U, D = 4, 4, 8, 8
KH = KW = 3
NIJ = KH * KW
H = W = 64
C = 32
N_FULL = 16
N_CORES = 8
IMG_PER_CORE = N_FULL // N_CORES
HP, WP = H + 2, W + 2              # padded input
LT_ROWS = 2                        # output rows per 128-loc tile
NLT = H // LT_ROWS                 # 32 loc-tiles per image
ST_LT = 4                          # loc-tiles per super-tile (512 locs)
NST = NLT // ST_LT                 # 8 super-tiles per image
PB = 2                             # super-tiles batched per routing pass
PLT = PB * ST_LT                   # loc-tiles per routing pass (8)
EPS = 1e-12

f32 = mybir.dt.float32
AL = mybir.AluOpType
AF = mybir.ActivationFunctionType
AX = mybir.AxisListType

_COMPILED = None


def _build(dump=False, repeat=1):
    nc = bacc.Bacc("TRN2", target_bir_lowering=False, debug=False)

    dbg = {}
    if dump:
        for name, shape in [("dbg_P", [128, ST_LT * 1152]),
                            ("dbg_s0", [128, ST_LT * 32]),
                            ("dbg_v0", [128, ST_LT * 32]),
                            ("dbg_b1", [128, ST_LT * 144]),
                            ("dbg_E1", [128, ST_LT * 144]),
                            ("dbg_s1", [128, ST_LT * 32])]:
            dbg[name] = nc.dram_tensor(name, shape, f32,
                                       kind="ExternalOutput").ap()

    xin_d = nc.dram_tensor("xin", [IMG_PER_CORE, C, H * W], f32,
                           kind="ExternalInput").ap()
    wmov_d = nc.dram_tensor("wmov", [C, NIJ * 128], f32,
                            kind="ExternalInput").ap()
    wsum_d = nc.dram_tensor("wsum", [C, NIJ * 32], f32,
                            kind="ExternalInput").ap()
    ident_d = nc.dram_tensor("ident", [128, 128], f32,
                             kind="ExternalInput").ap()
    out_d = nc.dram_tensor("out", [IMG_PER_CORE, C, H * W], f32,
                           kind="ExternalOutput").ap()

    with tile.TileContext(nc) as tc:
        with tc.tile_pool(name="const", bufs=1) as cpool, \
             tc.tile_pool(name="xpad", bufs=1) as xpool, \
             tc.tile_pool(name="stage", bufs=1) as spool, \
             tc.tile_pool(name="pst", bufs=4) as ppool, \
             tc.tile_pool(name="gh", bufs=2) as ghpool, \
             tc.tile_pool(name="small", bufs=2) as smpool, \
             tc.tile_pool(name="ppri", bufs=2, space="PSUM") as ppri, \
             tc.tile_pool(name="ps0", bufs=1, space="PSUM") as ps0, \
             tc.tile_pool(name="ptp", bufs=1, space="PSUM") as ptp:

            wmov_s = cpool.tile([C, NIJ * 128], f32, tag="wmov")
            wsum_s = cpool.tile([C, NIJ * 32], f32, tag="wsum")
            ident_s = cpool.tile([128, 128], f32, tag="ident")
            eps_s = cpool.tile([128, 1], f32, tag="eps")
            nc.sync.dma_start(wmov_s[:], wmov_d[:])
            nc.sync.dma_start(wsum_s[:], wsum_d[:])
            nc.sync.dma_start(ident_s[:], ident_d[:])
            nc.gpsimd.memset(eps_s[:], EPS)

            for img in range(IMG_PER_CORE):
                # one shared xpad slot: image n+1's fill overlaps image n's
                # routing tail (P-production finishes ~100us early)
                xp = xpool.tile([C, HP * WP], f32, tag="xpad")
                nc.gpsimd.memset(xp[:], 0.0)
                xv = xp[:].rearrange("p (h w) -> p h w", h=HP, w=WP)
                nc.sync.dma_start(
                    xv[:, 1:1 + H, 1:1 + W],
                    xin_d[img].rearrange("p (h w) -> p h w", h=H, w=W))
                xpads = {img: xv}
                stage = spool.tile([C, H * W], f32, tag="stage")
                for pr_rep in range((NST // PB) * repeat):
                    pr = pr_rep % (NST // PB)
                    # produce priors for PB super-tiles; small per-location
                    # tensors are batched across the pair to amortize DVE
                    # per-instruction overhead.
                    P_sts = []
                    s0_st = smpool.tile([128, PLT * 32], f32, tag="s0")
                    for half in range(PB):
                        st = pr * PB + half
                        P_st = ppool.tile([128, ST_LT * 1152], f32, tag="P")
                        P_sts.append(P_st)
                        for lt in range(ST_LT):
                            r0 = (st * ST_LT + lt) * LT_ROWS
                            glt = half * ST_LT + lt
                            pp = ppri.tile([128, 1152], f32, tag="ppri")
                            s0p = ps0.tile([128, 32], f32, tag="s0p")
                            for ij in range(NIJ):
                                i, j = ij // KW, ij % KW
                                for r in range(LT_ROWS):
                                    xw = xpads[img][:, r0 + i + r, j:j + W]
                                    prow = slice(r * W, (r + 1) * W)
                                    nc.tensor.matmul(
                                        pp[prow, ij * 128:(ij + 1) * 128],
                                        xw,
                                        wmov_s[:, ij * 128:(ij + 1) * 128],
                                        start=True, stop=True)
                                    nc.tensor.matmul(
                                        s0p[prow], xw,
                                        wsum_s[:, ij * 32:(ij + 1) * 32],
                                        start=(ij == 0),
                                        stop=(ij == NIJ - 1))
                            nc.scalar.copy(
                                P_st[:, lt * 1152:(lt + 1) * 1152], pp[:])
                            nc.scalar.copy(
                                s0_st[:, glt * 32:(glt + 1) * 32], s0p[:])

                    # ------- routing on this super-tile pair -------
                    def P5(half, lt):
                        return P_sts[half][:, lt * 1152:(lt + 1) *
                                           1152].rearrange(
                            "p (ij o u f) -> p ij o u f", ij=NIJ, o=O, u=U,
                            f=F)

                    def squash(s_st, tagp, newton=True):
                        # s_st: [128, (lt, o, u)]; returns v [128, (lt,o,u)]
                        # sqrt & reciprocals go through the ACT exp/ln
                        # tables (single table set, no DVE reciprocal);
                        # one Newton step restores sqrt to fp32 accuracy.
                        # newton=False skips it where the ~5e-6 table error
                        # is not amplified (the final squash: error passes
                        # straight to the output instead of through the
                        # routing logits).
                        sq = smpool.tile([128, PLT * 32], f32,
                                         tag=f"sq{tagp}")
                        nc.scalar.activation(sq[:], s_st[:], AF.Square)
                        n2 = smpool.tile([128, PLT * O], f32,
                                         tag=f"n2{tagp}")
                        nc.vector.tensor_reduce(
                            n2[:],
                            sq[:].rearrange("p (lt o u) -> p lt o u",
                                            lt=PLT, o=O, u=U),
                            AX.X, AL.add)
                        Ltile = smpool.tile([128, PLT * O], f32,
                                            tag=f"L{tagp}")
                        nc.scalar.activation(Ltile[:], n2[:], AF.Ln,
                                             bias=eps_s[:])
                        t_ = smpool.tile([128, PLT * O], f32,
                                         tag=f"t{tagp}")
                        nc.scalar.activation(t_[:], Ltile[:], AF.Exp,
                                             scale=0.5)
                        if newton:
                            r5 = smpool.tile([128, PLT * O], f32,
                                             tag=f"r5{tagp}")
                            # true Newton needs 1/t of the current t —
                            # exact DVE reciprocal (exp(-ln t) tables are
                            # ~1e-5 off)
                            nc.vector.reciprocal(r5[:], t_[:])
                            xr = smpool.tile([128, PLT * O], f32,
                                             tag=f"xr{tagp}")
                            # xr = n2 * (0.5/t)  (eps negligible: fi -> 0
                            # as n2 -> 0 regardless)
                            nc.vector.scalar_tensor_tensor(
                                xr[:], r5[:], 0.5, n2[:], AL.mult, AL.mult)
                            # t = 0.5*t + xr   (Newton)
                            nc.vector.scalar_tensor_tensor(
                                t_[:], t_[:], 0.5, xr[:], AL.mult, AL.add)
                        # w = (1+n2)*t;  fi = n2 / w
                        pw = smpool.tile([128, PLT * O], f32,
                                         tag=f"pw{tagp}")
                        nc.vector.scalar_tensor_tensor(
                            pw[:], n2[:], 1.0, t_[:], AL.add, AL.mult)
                        rw = smpool.tile([128, PLT * O], f32,
                                         tag=f"rw{tagp}")
                        nc.vector.reciprocal(rw[:], pw[:])
                        fi = smpool.tile([128, PLT * O], f32,
                                         tag=f"fi{tagp}")
                        nc.vector.tensor_tensor(fi[:], n2[:], rw[:], AL.mult)
                        v = smpool.tile([128, PLT * 32], f32,
                                        tag=f"v{tagp}")
                        fib = fi[:].rearrange("p (lt o) -> p lt o",
                                              lt=PLT).unsqueeze(3)
                        nc.vector.tensor_tensor(
                            v[:].rearrange("p (lt o u) -> p lt o u",
                                           lt=PLT, o=O, u=U),
                            s_st[:].rearrange("p (lt o u) -> p lt o u",
                                              lt=PLT, o=O, u=U),
                            fib.broadcast_to((128, PLT, O, U)), AL.mult)
                        return v

                    is_dbg = dump and img == 0 and pr == 0
                    if is_dbg:
                        nc.sync.dma_start(dbg["dbg_P"][:], P_sts[0][:])
                        nc.sync.dma_start(dbg["dbg_s0"][:],
                                          s0_st[:, :ST_LT * 32])

                    v = squash(s0_st, "0")
                    if is_dbg:
                        nc.sync.dma_start(dbg["dbg_v0"][:],
                                          v[:, :ST_LT * 32])

                    # b1[l, (half, lt, ij, o, f)] = sum_u P * v0
                    b_st = smpool.tile([128, PLT * 144], f32, tag="b")
                    hred = smpool.tile([128, PLT * 144], f32, tag="hred")
                    for it in range(3):
                        if it > 0:
                            # E = exp(b); Z = sum_o E; E' = E / Z
                            E = smpool.tile([128, PLT * 144], f32, tag="E")
                            nc.scalar.activation(E[:], b_st[:], AF.Exp)
                            Ev = E[:].rearrange(
                                "p (lt ij o f) -> p lt ij o f", lt=PLT,
                                ij=NIJ, o=O, f=F)
                            Z = smpool.tile([128, PLT * 36], f32, tag="Z")
                            nc.vector.tensor_reduce(
                                Z[:], Ev.transpose([0, 1, 2, 4, 3]), AX.X,
                                AL.add)
                            Zi = smpool.tile([128, PLT * 36], f32,
                                             tag="Zi")
                            nc.vector.reciprocal(Zi[:], Z[:])
                            Zib = Zi[:].rearrange(
                                "p (lt ij f) -> p lt ij f", lt=PLT,
                                ij=NIJ).unsqueeze(3).broadcast_to(
                                    (128, PLT, NIJ, O, F))
                            nc.vector.tensor_tensor(Ev, Ev, Zib, AL.mult)
                            s_st = smpool.tile([128, PLT * 32], f32,
                                               tag="s")
                            KK = ST_LT * NIJ
                            for half in range(PB):
                                # G = E' * P, one op per half ((lt, ij)
                                # collapses to one affine axis k)
                                G = ghpool.tile([128, ST_LT * 1152], f32,
                                                tag="gh")
                                Gk = G[:].rearrange(
                                    "p (k o u f) -> p k o u f", k=KK, o=O,
                                    u=U, f=F)
                                Pk = P_sts[half][:].rearrange(
                                    "p (k o u f) -> p k o u f", k=KK, o=O,
                                    u=U, f=F)
                                Ek = E[:, half * ST_LT * 144:(half + 1) *
                                       ST_LT * 144].rearrange(
                                    "p (k o f) -> p k o f", k=KK,
                                    o=O).unsqueeze(3).broadcast_to(
                                        (128, KK, O, U, F))
                                nc.vector.tensor_tensor(Gk, Pk, Ek, AL.mult)
                                # s[l, (half, lt, o, u)] = sum_{ij,f} G
                                for lt in range(ST_LT):
                                    glt = half * ST_LT + lt
                                    G5 = G[:, lt * 1152:(lt + 1) *
                                           1152].rearrange(
                                               "p (ij o u f) -> p ij o u f",
                                               ij=NIJ, o=O, u=U, f=F)
                                    nc.vector.tensor_reduce(
                                        s_st[:, glt * 32:(glt + 1) * 32],
                                        G5.transpose([0, 2, 3, 1, 4]),
                                        AX.XY, AL.add)
                            if is_dbg and it == 1:
                                nc.sync.dma_start(dbg["dbg_E1"][:],
                                                  E[:, :ST_LT * 144])
                                nc.sync.dma_start(dbg["dbg_s1"][:],
                                                  s_st[:, :ST_LT * 32])
                            v = squash(s_st, "12")
                        if it < 2:
                            # accumulate logits: b += sum_u P * v
                            dst = b_st if it == 0 else hred
                            for half in range(PB):
                                Hst = ghpool.tile([128, ST_LT * 1152], f32,
                                                  tag="gh")
                                # H = P * v_bcast per lt (v's broadcast AP
                                # needs [ij, (o,u), f] = 3 AP dims; adding
                                # lt would exceed the DVE TENSOR3D limit).
                                for lt in range(ST_LT):
                                    glt = half * ST_LT + lt
                                    H5 = Hst[:, lt * 1152:(lt + 1) *
                                             1152].rearrange(
                                                 "p (ij o u f) -> "
                                                 "p ij o u f",
                                                 ij=NIJ, o=O, u=U, f=F)
                                    vb = v[:, glt * 32:(glt + 1) *
                                           32].rearrange(
                                        "p (o u) -> p o u",
                                        o=O).unsqueeze(1).unsqueeze(
                                            4).broadcast_to(
                                                (128, NIJ, O, U, F))
                                    nc.vector.tensor_tensor(
                                        H5, P5(half, lt), vb, AL.mult)
                                # one segmented reduce over u per half
                                Hk = Hst[:].rearrange(
                                    "p (k o u f) -> p k o u f",
                                    k=ST_LT * NIJ, o=O, u=U, f=F)
                                nc.vector.tensor_reduce(
                                    dst[:, half * ST_LT * 144:(half + 1) *
                                        ST_LT * 144],
                                    Hk.transpose([0, 1, 2, 4, 3]),
                                    AX.X, AL.add)
                            if it == 0 and is_dbg:
                                nc.sync.dma_start(dbg["dbg_b1"][:],
                                                  b_st[:, :ST_LT * 144])
                            if it == 1:
                                nc.vector.tensor_tensor(b_st[:], b_st[:],
                                                        hred[:], AL.add)

                    # v now holds squash(s2): transpose to [32, locs] & stage
                    for glt in range(PLT):
                        r0 = (pr * PLT + glt) * LT_ROWS
                        tp = ptp.tile([32, 128], f32, tag="tp")
                        nc.tensor.transpose(tp[:],
                                            v[:, glt * 32:(glt + 1) * 32],
                                            ident_s[:])
                        nc.scalar.copy(
                            stage[:, r0 * W:r0 * W + LT_ROWS * W], tp[:])

                nc.sync.dma_start(out_d[img], stage[:])

    nc.compile()
    return nc


def _get_compiled():
    global _COMPILED
    if _COMPILED is None:
        _COMPILED = _build()
    return _COMPILED


def _make_consts(weight):
    w = np.asarray(weight, dtype=np.float32)  # [o, f, i, j, u, d]
    wmov = np.zeros((C, NIJ * 128), dtype=np.float32)
    wsum = np.zeros((C, NIJ * 32), dtype=np.float32)
    for o in range(O):
        for f in range(F):
            for ij in range(NIJ):
                i, j = ij // KW, ij % KW
                for u in range(U):
                    for d in range(D):
                        wmov[f * D + d,
                             ij * 128 + o * 32 + u * 4 + f] = w[o, f, i, j,
                                                                u, d]
                        wsum[f * D + d,
                             ij * 32 + o * 8 + u] = 0.25 * w[o, f, i, j, u,
                                                             d]
    return wmov, wsum


def kernel(x, weight):
    x = np.ascontiguousarray(np.asarray(x, dtype=np.float32))
    wmov, wsum = _make_consts(weight)
    ident = np.eye(128, dtype=np.float32)

    nc = _get_compiled()
    in_maps = []
    for c in range(N_CORES):
        xin = x[c * IMG_PER_CORE:(c + 1) * IMG_PER_CORE].reshape(
            IMG_PER_CORE, C, H * W)
        in_maps.append({
            "xin": np.ascontiguousarray(xin),
            "wmov": wmov,
            "wsum": wsum,
            "ident": ident,
        })
    res = bass_utils.run_bass_kernel_spmd(nc, in_maps,
                                          core_ids=list(range(N_CORES)))
    out = np.empty((N_FULL, C, H, W), dtype=np.float32)
    for c in range(N_CORES):
        out[c * IMG_PER_CORE:(c + 1) * IMG_PER_CORE] = res.results[c][
            "out"].reshape(IMG_PER_CORE, C, H, W)
    return out

